# revision 13
# baseline (speedup 1.0000x reference)
"""Trainium2 Bass kernel for nn_PSN (gnn_message_passing), 8 NeuronCores.

Math (per reference):
    deg  = segment_sum(ones, col); deg[deg==0] = 1; dis = deg^-1/2
    repeat L times:  agg = scatter_add(col, dis[row]*dis[col]*cur[row]);
                     cur = cur - agg; update += tanh(k_i)*cur
    c = sigmoid(weighting); h = c*update + (1-c)*x; out = relu(h @ W.T + b)

Strategy (v4, quadrant-pipelined block-dense SpMM): target nodes are
sharded across 8 cores (1280/core); the adjacency is a dense grid of
128x128 count blocks in fp8 (exact small ints) streamed from HBM -- no
per-edge DMA.  Both the target dim and the source dim are split in half
and the layer loop is software-pipelined so each half-AllGather for the
next layer runs underneath the other half's matmuls:

  per layer:  MM(T0,S0) MM(T0,S1) | drain T0, y'_T0, AllGather_A(next)
              MM(T1,S0) MM(T1,S1) | drain T1, y'_T1, AllGather_B(next)

State is feature-major (curT [128 f, 1280 n]) so the final linear layer
needs no transposes; per-node scaling uses a pre-expanded disxT tensor.
"""
import sys
import types
import numpy as np
import ml_dtypes
from contextlib import ExitStack

import concourse.bass as bass
import concourse.tile as tile
from concourse import bacc, mybir
from concourse.bass_utils import run_bass_kernel_spmd

P = 128          # partitions / block size
NC = 8           # cores
F = 128          # feature dim (must equal P)
L = 8            # layers

FP32 = mybir.dt.float32
BF16 = mybir.dt.bfloat16
FP8 = mybir.dt.float8e4

LAST_EXEC_TIME_NS = None


def _install_ntff_hook():
    try:
        if "antenv.axon_hooks" in sys.modules:
            return
        import antenv
        from trn_agent_boot.trn_boot import _ntff_profile_via_ctypes

        m = types.ModuleType("antenv.axon_hooks")
        _state = {"hook": _ntff_profile_via_ctypes("/opt/axon/libaxon_pjrt.so")}
        m.set_axon_ntff_profile_hook = lambda h: _state.__setitem__("hook", h)
        m.get_axon_ntff_profile_hook = lambda: _state["hook"]
        sys.modules["antenv.axon_hooks"] = m
        antenv.axon_hooks = m
    except Exception:
        pass


def preprocess(x, edge_index):
    """Host-side index/layout prep: quadrant-ordered block-count adjacency
    (fp8), degree histogram, feature-major x slices."""
    N = x.shape[0]
    NB = -(-N // (NC * P))           # target blocks per core
    assert NB % 2 == 0
    NPC = NB * P
    NTOT = NC * NPC
    RB = NTOT // P
    HB = NB // 2
    RBH = RB // 2
    NPCh = NPC // 2

    row = np.asarray(edge_index[0], dtype=np.int64)
    col = np.asarray(edge_index[1], dtype=np.int64)

    A2 = np.zeros((NTOT, NTOT), dtype=np.int16)
    np.add.at(A2, (row, col), 1)

    deg = np.zeros(NTOT, dtype=np.float32)
    deg[:N] = np.bincount(col, minlength=N).astype(np.float32)

    x_pad = np.zeros((NTOT, F), dtype=np.float32)
    x_pad[:N] = np.asarray(x, dtype=np.float32)

    # source-block order: all ranks' first half-slices, then second halves
    rb_order = [r * NB + b for r in range(NC) for b in range(HB)] + \
               [r * NB + b for r in range(NC) for b in range(HB, NB)]

    per_core = []
    for r in range(NC):
        cs = slice(r * NPC, (r + 1) * NPC)
        a_blocks = A2[:, cs].reshape(RB, P, NPC)[rb_order]  # [rb, p, tgt]
        # [t-half, s-half, rb_local, p, tgt_local]
        a_q = np.ascontiguousarray(
            a_blocks.reshape(2, RBH, P, 2, NPCh).transpose(3, 0, 1, 2, 4))
        live = [[[rl for rl in range(RBH) if a_q[t, s, rl].any()]
                 for s in range(2)] for t in range(2)]
        a_r = a_q.astype(ml_dtypes.float8_e4m3)
        xsT = np.ascontiguousarray(x_pad[cs].T)            # [F, NPC]
        deg_nm = np.ascontiguousarray(deg[cs].reshape(NB, P).T)
        deg_row = np.ascontiguousarray(deg[cs].reshape(1, NPC))
        per_core.append({"a": a_r, "xsT": xsT, "deg_nm": deg_nm,
                         "deg_row": deg_row, "live": live})
    return per_core, NB, NPC, NTOT, N


def build_program(NB, live):
    """live[t][s] = list of non-zero source blocks (intersected over cores
    it must be a superset; we use the union so data is correct)."""
    NPC = NB * P
    NTOT = NC * NPC
    RB = NTOT // P
    HB = NB // 2
    RBH = RB // 2
    NPCh = NPC // 2
    HROWS = NC * HB * P
    RB_CHUNK = 8
    gsz = [512, NPCh - 512] if NPCh > 512 else [NPCh]
    ngr = len(gsz)

    nc = bacc.Bacc("TRN2", target_bir_lowering=False, debug=False,
                   enable_asserts=False, num_devices=NC)

    a_in = nc.dram_tensor("a", [2, 2, RBH, P, NPCh], FP8, kind="ExternalInput")
    x_in = nc.dram_tensor("xsT", [F, NPC], FP32, kind="ExternalInput")
    dnm_in = nc.dram_tensor("deg_nm", [P, NB], FP32, kind="ExternalInput")
    drow_in = nc.dram_tensor("deg_row", [1, NPC], FP32, kind="ExternalInput")
    wt_in = nc.dram_tensor("wt", [F, F], FP32, kind="ExternalInput")      # W.T
    bias_in = nc.dram_tensor("bias", [F, 1], FP32, kind="ExternalInput")
    kv_in = nc.dram_tensor("kv", [1, L], FP32, kind="ExternalInput")
    wg_in = nc.dram_tensor("wg", [1, 1], FP32, kind="ExternalInput")
    id_in = nc.dram_tensor("ident", [P, P], FP32, kind="ExternalInput")
    out_t = nc.dram_tensor("outT", [F, NPC], FP32, kind="ExternalOutput")

    with tile.TileContext(nc) as tc, ExitStack() as ctx:
        per = ctx.enter_context(tc.tile_pool(name="per", bufs=1))
        ap_pool = ctx.enter_context(tc.tile_pool(name="ap", bufs=4))
        wk = ctx.enter_context(tc.tile_pool(name="wk", bufs=2))
        ps_agg = ctx.enter_context(tc.tile_pool(name="ps_agg", bufs=1, space="PSUM"))
        ps_m = ctx.enter_context(tc.tile_pool(name="ps_m", bufs=2, space="PSUM"))
        ps_f = ctx.enter_context(tc.tile_pool(name="ps_f", bufs=2, space="PSUM"))
        dram = ctx.enter_context(tc.tile_pool(name="dram", bufs=1, space="DRAM"))

        # persistent state (feature-major)
        xsT = per.tile([F, NPC], FP32)
        curT = per.tile([F, NPC], FP32)
        updT = per.tile([F, NPC], FP32)
        disxT = per.tile([F, NPC], FP32)
        ndisxT = per.tile([F, NPC], FP32)
        dis_nm = per.tile([P, NB], FP32)
        y_sbA0 = per.tile([P, RBH * F], BF16)
        y_sbA1 = per.tile([P, RBH * F], BF16)
        y_sbB0 = per.tile([P, RBH * F], BF16)
        y_sbB1 = per.tile([P, RBH * F], BF16)
        y_sbAs = [y_sbA0, y_sbA1]
        y_sbBs = [y_sbB0, y_sbB1]
        yn = per.tile([P, NB * F], BF16)
        tanhk = per.tile([P, L], FP32)
        cbc = per.tile([P, 1], FP32)
        wt_sb = per.tile([F, F], FP32)
        id_sb = per.tile([P, P], FP32)
        bias_sb = per.tile([F, 1], FP32)
        ones1 = per.tile([1, P], FP32)
        outsb = per.tile([F, NPC], FP32)

        y_in = dram.tile([NPC, F], BF16)
        y_outA = nc.dram_tensor("y_outA_sh", [HROWS, F], BF16, addr_space="Shared").ap()
        y_outB = nc.dram_tensor("y_outB_sh", [HROWS, F], BF16, addr_space="Shared").ap()

        # ---- loads ----
        nc.sync.dma_start(xsT[:], x_in[:])
        nc.sync.dma_start(dis_nm[:], dnm_in[:])   # holds deg for now
        nc.sync.dma_start(wt_sb[:], wt_in[:])
        nc.sync.dma_start(id_sb[:], id_in[:])
        nc.sync.dma_start(bias_sb[:], bias_in[:])
        kv_sb = wk.tile([1, L], FP32)
        wg_sb = wk.tile([1, 1], FP32)
        drow = wk.tile([1, NPC], FP32)
        nc.sync.dma_start(kv_sb[:], kv_in[:])
        nc.sync.dma_start(wg_sb[:], wg_in[:])
        nc.sync.dma_start(drow[:], drow_in[:])

        nc.vector.memset(ones1[:], 1.0)
        nc.vector.memset(updT[:], 0.0)
        nc.vector.tensor_copy(curT[:], xsT[:])

        # ---- dis = (max(deg,1))^-1/2, both layouts ----
        nc.vector.tensor_scalar_max(dis_nm[:], dis_nm[:], 1.0)
        nc.vector.reciprocal(dis_nm[:], dis_nm[:])
        nc.scalar.activation(dis_nm[:], dis_nm[:], mybir.ActivationFunctionType.Sqrt)

        nc.vector.tensor_scalar_max(drow[:], drow[:], 1.0)
        nc.vector.reciprocal(drow[:], drow[:])
        nc.scalar.activation(drow[:], drow[:], mybir.ActivationFunctionType.Sqrt)
        for g in range(-(-NPC // 512)):
            w = min(512, NPC - g * 512)
            psd = ps_m.tile([P, 512], FP32, space="PSUM", tag="misc")
            nc.tensor.matmul(psd[:, :w], ones1[:],
                             drow[:, g * 512:g * 512 + w], start=True, stop=True)
            nc.vector.tensor_copy(disxT[:, g * 512:g * 512 + w], psd[:, :w])
        nc.vector.tensor_scalar_mul(ndisxT[:], disxT[:], -1.0)

        # ---- broadcast tanh(k) and sigmoid(weighting) ----
        psb = ps_m.tile([P, 512], FP32, space="PSUM", tag="misc")
        nc.tensor.matmul(psb[:, :L], ones1[:], kv_sb[:], start=True, stop=True)
        nc.scalar.activation(tanhk[:], psb[:, :L], mybir.ActivationFunctionType.Tanh)
        psb1 = ps_m.tile([P, 512], FP32, space="PSUM", tag="misc")
        nc.tensor.matmul(psb1[:, :1], ones1[:], wg_sb[:], start=True, stop=True)
        nc.scalar.activation(cbc[:], psb1[:, :1], mybir.ActivationFunctionType.Sigmoid)

        def yprod_and_ag(t, y_outH):
            """transpose+scale own target half t of curT into yn, DMA to
            y_in half, AllGather into y_outH."""
            for b in range(t * HB, (t + 1) * HB):
                pst = ps_f.tile([P, 512], FP32, space="PSUM", tag="fin")
                nc.tensor.transpose(pst[:, :P], curT[:, b * F:(b + 1) * F], id_sb[:])
                nc.vector.tensor_scalar_mul(
                    yn[:, b * F:(b + 1) * F], pst[:, :P], dis_nm[:, b:b + 1])
            hs = slice(t * HB * P, (t + 1) * HB * P)
            nc.sync.dma_start(
                y_in[hs, :].rearrange("(b p) f -> p b f", p=P),
                yn[:, t * HB * F:(t + 1) * HB * F].rearrange("p (b f) -> p b f", f=F))
            nc.gpsimd.collective_compute(
                "AllGather", mybir.AluOpType.bypass,
                replica_groups=[list(range(NC))],
                ins=[y_in[hs, :].opt()], outs=[y_outH[:].opt()])

        def load_ysb(y_sbH, y_outH):
            nc.sync.dma_start(
                y_sbH[:].rearrange("p (rb f) -> p rb f", f=F),
                y_outH[:].rearrange("(rb p) f -> p rb f", p=P))

        # ---- initial y + AGs (parity 0) ----
        yprod_and_ag(0, y_outA)
        yprod_and_ag(1, y_outB)
        load_ysb(y_sbAs[0], y_outA)
        load_ysb(y_sbBs[0], y_outB)

        # ---- layers (quadrant pipeline) ----
        for i in range(L):
            for t in range(2):
                toff = t * NPCh
                psg = []
                for _g in range(ngr):
                    psgt = ps_agg.tile([P, 512], FP32, space="PSUM",
                                       tag=f"agg{t}{_g}")
                    psg.append(psgt)
                seq = [(0, rl) for rl in live[t][0]] + [(1, rl) for rl in live[t][1]]
                for qi, (s, rl) in enumerate(seq):
                    y_sbH = y_sbAs[i % 2] if s == 0 else y_sbBs[i % 2]
                    rc = rl // RB_CHUNK
                    if qi == 0 or (seq[qi - 1][0], seq[qi - 1][1] // RB_CHUNK) != (s, rc):
                        at = ap_pool.tile([P, RB_CHUNK * NPCh], FP8, tag="at")
                        nc.sync.dma_start(
                            at[:].rearrange("p (rb n) -> p rb n", n=NPCh),
                            a_in[t, s, rc * RB_CHUNK:(rc + 1) * RB_CHUNK, :, :]
                            .rearrange("rb p n -> p rb n"))
                    for g in range(ngr):
                        go = sum(gsz[:g])
                        nc.tensor.matmul(
                            psg[g][:, :gsz[g]],
                            y_sbH[:, rl * F:(rl + 1) * F],
                            at[:, rl % RB_CHUNK * NPCh + go:
                               rl % RB_CHUNK * NPCh + go + gsz[g]],
                            start=(qi == 0), stop=(qi == len(seq) - 1))
                # drain target half t:  curT += psum * (-disxT)
                for g in range(ngr):
                    sl = slice(toff + sum(gsz[:g]), toff + sum(gsz[:g]) + gsz[g])
                    tmp = wk.tile([P, 512], FP32, tag="tmp")
                    nc.vector.tensor_tensor(tmp[:, :gsz[g]], psg[g][:, :gsz[g]],
                                            ndisxT[:, sl], mybir.AluOpType.mult)
                    nc.vector.tensor_tensor(curT[:, sl], curT[:, sl],
                                            tmp[:, :gsz[g]], mybir.AluOpType.add)
                ths = slice(toff, toff + NPCh)
                nc.vector.scalar_tensor_tensor(
                    updT[:, ths], curT[:, ths], tanhk[:, i:i + 1], updT[:, ths],
                    op0=mybir.AluOpType.mult, op1=mybir.AluOpType.add)
                if i < L - 1:
                    # next layer's half-AG + reload, hidden under other half's MMs
                    y_outH = y_outA if t == 0 else y_outB
                    y_sbH = (y_sbAs if t == 0 else y_sbBs)[(i + 1) % 2]
                    yprod_and_ag(t, y_outH)
                    load_ysb(y_sbH, y_outH)

        # ---- epilogue ----
        t1 = per.tile([F, NPC], FP32)
        nc.vector.tensor_tensor(t1[:], updT[:], xsT[:], mybir.AluOpType.subtract)
        h = per.tile([F, NPC], FP32)
        nc.vector.scalar_tensor_tensor(
            h[:], t1[:], cbc[:, 0:1], xsT[:],
            op0=mybir.AluOpType.mult, op1=mybir.AluOpType.add)
        for g in range(-(-NPC // 512)):
            w = min(512, NPC - g * 512)
            psf = ps_f.tile([P, 512], FP32, space="PSUM", tag="fin")
            nc.tensor.matmul(psf[:, :w], wt_sb[:],
                             h[:, g * 512:g * 512 + w], start=True, stop=True)
            nc.scalar.activation(outsb[:, g * 512:g * 512 + w], psf[:, :w],
                                 mybir.ActivationFunctionType.Relu,
                                 bias=bias_sb[:, 0:1])
        nc.sync.dma_start(out_t[:], outsb[:])

    nc.compile()
    return nc


def kernel(x, edge_index, k_values, weighting, W, b):
    global LAST_EXEC_TIME_NS
    import os
    x = np.asarray(x)
    per_core, NB, NPC, NTOT, N = preprocess(x, edge_index)
    # a block may be skipped only if zero on every core
    live = [[sorted(set.union(*[set(pc["live"][t][s]) for pc in per_core]))
             for s in range(2)] for t in range(2)]
    nc = build_program(NB, live)

    wt = np.ascontiguousarray(np.asarray(W, dtype=np.float32).T)
    bias = np.asarray(b, dtype=np.float32).reshape(F, 1)
    kv = np.asarray(k_values, dtype=np.float32).reshape(1, L)
    wg = np.asarray(weighting, dtype=np.float32).reshape(1, 1)
    ident = np.eye(P, dtype=np.float32)

    in_maps = [
        {"a": pc["a"], "xsT": pc["xsT"], "deg_nm": pc["deg_nm"],
         "deg_row": pc["deg_row"],
         "wt": wt, "bias": bias, "kv": kv, "wg": wg, "ident": ident}
        for pc in per_core
    ]

    if os.environ.get("BASS_TRACE"):
        _install_ntff_hook()
    res = run_bass_kernel_spmd(nc, in_maps, core_ids=list(range(NC)))
    LAST_EXEC_TIME_NS = res.exec_time_ns

    out = np.empty((N, F), dtype=np.float32)
    for r in range(NC):
        lo = r * NPC
        hi = min(N, lo + NPC)
        if hi > lo:
            out[lo:hi] = res.results[r]["outT"].T[: hi - lo]
    return out


# revision 14
# speedup vs baseline: 1.0860x; 1.0860x over previous
"""Trainium2 Bass kernel for nn_PSN (gnn_message_passing), 8 NeuronCores.

Math (per reference):
    deg  = segment_sum(ones, col); deg[deg==0] = 1; dis = deg^-1/2
    repeat L times:  agg = scatter_add(col, dis[row]*dis[col]*cur[row]);
                     cur = cur - agg; update += tanh(k_i)*cur
    c = sigmoid(weighting); h = c*update + (1-c)*x; out = relu(h @ W.T + b)

Strategy (v4, quadrant-pipelined block-dense SpMM): target nodes are
sharded across 8 cores (1280/core); the adjacency is a dense grid of
128x128 count blocks in fp8 (exact small ints) streamed from HBM -- no
per-edge DMA.  Both the target dim and the source dim are split in half
and the layer loop is software-pipelined so each half-AllGather for the
next layer runs underneath the other half's matmuls:

  per layer:  MM(T0,S0) MM(T0,S1) | drain T0, y'_T0, AllGather_A(next)
              MM(T1,S0) MM(T1,S1) | drain T1, y'_T1, AllGather_B(next)

State is feature-major (curT [128 f, 1280 n]) so the final linear layer
needs no transposes; per-node scaling uses a pre-expanded disxT tensor.
"""
import sys
import types
import numpy as np
import ml_dtypes
from contextlib import ExitStack

import concourse.bass as bass
import concourse.tile as tile
from concourse import bacc, mybir
from concourse.bass_utils import run_bass_kernel_spmd

P = 128          # partitions / block size
NC = 8           # cores
F = 128          # feature dim (must equal P)
L = 8            # layers

FP32 = mybir.dt.float32
BF16 = mybir.dt.bfloat16
FP8 = mybir.dt.float8e4

LAST_EXEC_TIME_NS = None


def _install_ntff_hook():
    try:
        if "antenv.axon_hooks" in sys.modules:
            return
        import antenv
        from trn_agent_boot.trn_boot import _ntff_profile_via_ctypes

        m = types.ModuleType("antenv.axon_hooks")
        _state = {"hook": _ntff_profile_via_ctypes("/opt/axon/libaxon_pjrt.so")}
        m.set_axon_ntff_profile_hook = lambda h: _state.__setitem__("hook", h)
        m.get_axon_ntff_profile_hook = lambda: _state["hook"]
        sys.modules["antenv.axon_hooks"] = m
        antenv.axon_hooks = m
    except Exception:
        pass


def preprocess(x, edge_index):
    """Host-side index/layout prep: quadrant-ordered block-count adjacency
    (fp8), degree histogram, feature-major x slices."""
    N = x.shape[0]
    NB = -(-N // (NC * P))           # target blocks per core
    assert NB % 2 == 0
    NPC = NB * P
    NTOT = NC * NPC
    RB = NTOT // P
    HB = NB // 2
    RBH = RB // 2
    NPCh = NPC // 2

    row = np.asarray(edge_index[0], dtype=np.int64)
    col = np.asarray(edge_index[1], dtype=np.int64)

    A2 = np.zeros((NTOT, NTOT), dtype=np.int16)
    np.add.at(A2, (row, col), 1)

    deg = np.zeros(NTOT, dtype=np.float32)
    deg[:N] = np.bincount(col, minlength=N).astype(np.float32)

    x_pad = np.zeros((NTOT, F), dtype=np.float32)
    x_pad[:N] = np.asarray(x, dtype=np.float32)

    # source-block order: all ranks' first half-slices, then second halves
    rb_order = [r * NB + b for r in range(NC) for b in range(HB)] + \
               [r * NB + b for r in range(NC) for b in range(HB, NB)]

    per_core = []
    for r in range(NC):
        cs = slice(r * NPC, (r + 1) * NPC)
        a_blocks = A2[:, cs].reshape(RB, P, NPC)[rb_order]  # [rb, p, tgt]
        # [t-half, s-half, rb_local, p, tgt_local]
        a_q = np.ascontiguousarray(
            a_blocks.reshape(2, RBH, P, 2, NPCh).transpose(3, 0, 1, 2, 4))
        live = [[[rl for rl in range(RBH) if a_q[t, s, rl].any()]
                 for s in range(2)] for t in range(2)]
        a_r = a_q.astype(ml_dtypes.float8_e4m3)
        xsT = np.ascontiguousarray(x_pad[cs].T)            # [F, NPC]
        deg_nm = np.ascontiguousarray(deg[cs].reshape(NB, P).T)
        deg_row = np.ascontiguousarray(deg[cs].reshape(1, NPC))
        per_core.append({"a": a_r, "xsT": xsT, "deg_nm": deg_nm,
                         "deg_row": deg_row, "live": live})
    return per_core, NB, NPC, NTOT, N


def build_program(NB, live):
    """live[t][s] = list of non-zero source blocks (intersected over cores
    it must be a superset; we use the union so data is correct)."""
    NPC = NB * P
    NTOT = NC * NPC
    RB = NTOT // P
    HB = NB // 2
    RBH = RB // 2
    NPCh = NPC // 2
    HROWS = NC * HB * P
    RB_CHUNK = 8
    gsz = [512, NPCh - 512] if NPCh > 512 else [NPCh]
    ngr = len(gsz)

    nc = bacc.Bacc("TRN2", target_bir_lowering=False, debug=False,
                   enable_asserts=False, num_devices=NC)

    a_in = nc.dram_tensor("a", [2, 2, RBH, P, NPCh], FP8, kind="ExternalInput")
    x_in = nc.dram_tensor("xsT", [F, NPC], FP32, kind="ExternalInput")
    dnm_in = nc.dram_tensor("deg_nm", [P, NB], FP32, kind="ExternalInput")
    drow_in = nc.dram_tensor("deg_row", [1, NPC], FP32, kind="ExternalInput")
    wt_in = nc.dram_tensor("wt", [F, F], FP32, kind="ExternalInput")      # W.T
    bias_in = nc.dram_tensor("bias", [F, 1], FP32, kind="ExternalInput")
    kv_in = nc.dram_tensor("kv", [1, L], FP32, kind="ExternalInput")
    wg_in = nc.dram_tensor("wg", [1, 1], FP32, kind="ExternalInput")
    id_in = nc.dram_tensor("ident", [P, P], FP32, kind="ExternalInput")
    out_t = nc.dram_tensor("outT", [F, NPC], FP32, kind="ExternalOutput")

    with tile.TileContext(nc) as tc, ExitStack() as ctx:
        per = ctx.enter_context(tc.tile_pool(name="per", bufs=1))
        ap_pool = ctx.enter_context(tc.tile_pool(name="ap", bufs=4))
        wk = ctx.enter_context(tc.tile_pool(name="wk", bufs=2))
        ps_agg = ctx.enter_context(tc.tile_pool(name="ps_agg", bufs=1, space="PSUM"))
        ps_m = ctx.enter_context(tc.tile_pool(name="ps_m", bufs=2, space="PSUM"))
        ps_f = ctx.enter_context(tc.tile_pool(name="ps_f", bufs=2, space="PSUM"))
        dram = ctx.enter_context(tc.tile_pool(name="dram", bufs=1, space="DRAM"))

        # persistent state (feature-major)
        xsT = per.tile([F, NPC], FP32)
        curT = per.tile([F, NPC], FP32)
        updT = per.tile([F, NPC], FP32)
        disxT = per.tile([F, NPC], FP32)
        ndisxT = per.tile([F, NPC], FP32)
        dis_nm = per.tile([P, NB], FP32)
        y_sbA0 = per.tile([P, RBH * F], BF16)
        y_sbA1 = per.tile([P, RBH * F], BF16)
        y_sbB0 = per.tile([P, RBH * F], BF16)
        y_sbB1 = per.tile([P, RBH * F], BF16)
        y_sbAs = [y_sbA0, y_sbA1]
        y_sbBs = [y_sbB0, y_sbB1]
        yn = per.tile([P, NB * F], BF16)
        tanhk = per.tile([P, L], FP32)
        cbc = per.tile([P, 1], FP32)
        wt_sb = per.tile([F, F], FP32)
        id_sb = per.tile([P, P], FP32)
        bias_sb = per.tile([F, 1], FP32)
        ones1 = per.tile([1, P], FP32)
        outsb = per.tile([F, NPC], FP32)

        y_in = dram.tile([NPC, F], BF16)
        y_outA = nc.dram_tensor("y_outA_sh", [HROWS, F], BF16, addr_space="Shared").ap()
        y_outB = nc.dram_tensor("y_outB_sh", [HROWS, F], BF16, addr_space="Shared").ap()

        # ---- loads ----
        nc.sync.dma_start(xsT[:], x_in[:])
        nc.sync.dma_start(dis_nm[:], dnm_in[:])   # holds deg for now
        nc.sync.dma_start(wt_sb[:], wt_in[:])
        nc.sync.dma_start(id_sb[:], id_in[:])
        nc.sync.dma_start(bias_sb[:], bias_in[:])
        kv_sb = wk.tile([1, L], FP32)
        wg_sb = wk.tile([1, 1], FP32)
        drow = wk.tile([1, NPC], FP32)
        nc.sync.dma_start(kv_sb[:], kv_in[:])
        nc.sync.dma_start(wg_sb[:], wg_in[:])
        nc.sync.dma_start(drow[:], drow_in[:])

        nc.vector.memset(ones1[:], 1.0)
        nc.vector.memset(updT[:], 0.0)
        nc.vector.tensor_copy(curT[:], xsT[:])

        # ---- dis = (max(deg,1))^-1/2, both layouts ----
        nc.vector.tensor_scalar_max(dis_nm[:], dis_nm[:], 1.0)
        nc.vector.reciprocal(dis_nm[:], dis_nm[:])
        nc.scalar.activation(dis_nm[:], dis_nm[:], mybir.ActivationFunctionType.Sqrt)

        nc.vector.tensor_scalar_max(drow[:], drow[:], 1.0)
        nc.vector.reciprocal(drow[:], drow[:])
        nc.scalar.activation(drow[:], drow[:], mybir.ActivationFunctionType.Sqrt)
        for g in range(-(-NPC // 512)):
            w = min(512, NPC - g * 512)
            psd = ps_m.tile([P, 512], FP32, space="PSUM", tag="misc")
            nc.tensor.matmul(psd[:, :w], ones1[:],
                             drow[:, g * 512:g * 512 + w], start=True, stop=True)
            nc.vector.tensor_copy(disxT[:, g * 512:g * 512 + w], psd[:, :w])
        nc.vector.tensor_scalar_mul(ndisxT[:], disxT[:], -1.0)

        # ---- broadcast tanh(k) and sigmoid(weighting) ----
        psb = ps_m.tile([P, 512], FP32, space="PSUM", tag="misc")
        nc.tensor.matmul(psb[:, :L], ones1[:], kv_sb[:], start=True, stop=True)
        nc.scalar.activation(tanhk[:], psb[:, :L], mybir.ActivationFunctionType.Tanh)
        psb1 = ps_m.tile([P, 512], FP32, space="PSUM", tag="misc")
        nc.tensor.matmul(psb1[:, :1], ones1[:], wg_sb[:], start=True, stop=True)
        nc.scalar.activation(cbc[:], psb1[:, :1], mybir.ActivationFunctionType.Sigmoid)

        def yprod_and_ag(t, y_outH):
            """transpose+scale own target half t of curT into yn, DMA to
            y_in half, AllGather into y_outH."""
            for b in range(t * HB, (t + 1) * HB):
                pst = ps_f.tile([P, 512], FP32, space="PSUM", tag="fin")
                nc.tensor.transpose(pst[:, :P], curT[:, b * F:(b + 1) * F], id_sb[:])
                nc.vector.tensor_scalar_mul(
                    yn[:, b * F:(b + 1) * F], pst[:, :P], dis_nm[:, b:b + 1])
            hs = slice(t * HB * P, (t + 1) * HB * P)
            nc.sync.dma_start(
                y_in[hs, :].rearrange("(b p) f -> p b f", p=P),
                yn[:, t * HB * F:(t + 1) * HB * F].rearrange("p (b f) -> p b f", f=F))
            nc.gpsimd.collective_compute(
                "AllGather", mybir.AluOpType.bypass,
                replica_groups=[list(range(NC))],
                ins=[y_in[hs, :].opt()], outs=[y_outH[:].opt()])

        def load_ysb(y_sbH, y_outH):
            # chunked so matmuls can begin after the first chunk lands
            for c in range(RBH // RB_CHUNK):
                cw = RB_CHUNK * F
                nc.sync.dma_start(
                    y_sbH[:, c * cw:(c + 1) * cw].rearrange("p (rb f) -> p rb f", f=F),
                    y_outH[c * RB_CHUNK * P:(c + 1) * RB_CHUNK * P, :]
                    .rearrange("(rb p) f -> p rb f", p=P))

        # ---- initial y + AGs (parity 0) ----
        yprod_and_ag(0, y_outA)
        yprod_and_ag(1, y_outB)
        load_ysb(y_sbAs[0], y_outA)
        load_ysb(y_sbBs[0], y_outB)

        # ---- layers (quadrant pipeline, ping-pong half order) ----
        for i in range(L):
            t_first = i % 2
            s_first = 0 if i == 0 else (i - 1) % 2
            for t in (t_first, 1 - t_first):
                toff = t * NPCh
                psg = []
                for _g in range(ngr):
                    psgt = ps_agg.tile([P, 512], FP32, space="PSUM",
                                       tag=f"agg{t}{_g}")
                    psg.append(psgt)
                seq = [(s_first, rl) for rl in live[t][s_first]] + \
                      [(1 - s_first, rl) for rl in live[t][1 - s_first]]
                for qi, (s, rl) in enumerate(seq):
                    y_sbH = y_sbAs[i % 2] if s == 0 else y_sbBs[i % 2]
                    rc = rl // RB_CHUNK
                    if qi == 0 or (seq[qi - 1][0], seq[qi - 1][1] // RB_CHUNK) != (s, rc):
                        at = ap_pool.tile([P, RB_CHUNK * NPCh], FP8, tag="at")
                        nc.sync.dma_start(
                            at[:].rearrange("p (rb n) -> p rb n", n=NPCh),
                            a_in[t, s, rc * RB_CHUNK:(rc + 1) * RB_CHUNK, :, :]
                            .rearrange("rb p n -> p rb n"))
                    for g in range(ngr):
                        go = sum(gsz[:g])
                        nc.tensor.matmul(
                            psg[g][:, :gsz[g]],
                            y_sbH[:, rl * F:(rl + 1) * F],
                            at[:, rl % RB_CHUNK * NPCh + go:
                               rl % RB_CHUNK * NPCh + go + gsz[g]],
                            start=(qi == 0), stop=(qi == len(seq) - 1))
                # drain target half t:  curT += psum * (-disxT)
                for g in range(ngr):
                    sl = slice(toff + sum(gsz[:g]), toff + sum(gsz[:g]) + gsz[g])
                    tmp = wk.tile([P, 512], FP32, tag="tmp")
                    nc.vector.tensor_tensor(tmp[:, :gsz[g]], psg[g][:, :gsz[g]],
                                            ndisxT[:, sl], mybir.AluOpType.mult)
                    nc.vector.tensor_tensor(curT[:, sl], curT[:, sl],
                                            tmp[:, :gsz[g]], mybir.AluOpType.add)
                ths = slice(toff, toff + NPCh)
                nc.vector.scalar_tensor_tensor(
                    updT[:, ths], curT[:, ths], tanhk[:, i:i + 1], updT[:, ths],
                    op0=mybir.AluOpType.mult, op1=mybir.AluOpType.add)
                if i < L - 1:
                    # next layer's half-AG + reload, hidden under other half's MMs
                    y_outH = y_outA if t == 0 else y_outB
                    y_sbH = (y_sbAs if t == 0 else y_sbBs)[(i + 1) % 2]
                    yprod_and_ag(t, y_outH)
                    load_ysb(y_sbH, y_outH)

        # ---- epilogue ----
        t1 = per.tile([F, NPC], FP32)
        nc.vector.tensor_tensor(t1[:], updT[:], xsT[:], mybir.AluOpType.subtract)
        h = per.tile([F, NPC], FP32)
        nc.vector.scalar_tensor_tensor(
            h[:], t1[:], cbc[:, 0:1], xsT[:],
            op0=mybir.AluOpType.mult, op1=mybir.AluOpType.add)
        for g in range(-(-NPC // 512)):
            w = min(512, NPC - g * 512)
            psf = ps_f.tile([P, 512], FP32, space="PSUM", tag="fin")
            nc.tensor.matmul(psf[:, :w], wt_sb[:],
                             h[:, g * 512:g * 512 + w], start=True, stop=True)
            nc.scalar.activation(outsb[:, g * 512:g * 512 + w], psf[:, :w],
                                 mybir.ActivationFunctionType.Relu,
                                 bias=bias_sb[:, 0:1])
        nc.sync.dma_start(out_t[:], outsb[:])

    nc.compile()
    return nc


def kernel(x, edge_index, k_values, weighting, W, b):
    global LAST_EXEC_TIME_NS
    import os
    x = np.asarray(x)
    per_core, NB, NPC, NTOT, N = preprocess(x, edge_index)
    # a block may be skipped only if zero on every core
    live = [[sorted(set.union(*[set(pc["live"][t][s]) for pc in per_core]))
             for s in range(2)] for t in range(2)]
    nc = build_program(NB, live)

    wt = np.ascontiguousarray(np.asarray(W, dtype=np.float32).T)
    bias = np.asarray(b, dtype=np.float32).reshape(F, 1)
    kv = np.asarray(k_values, dtype=np.float32).reshape(1, L)
    wg = np.asarray(weighting, dtype=np.float32).reshape(1, 1)
    ident = np.eye(P, dtype=np.float32)

    in_maps = [
        {"a": pc["a"], "xsT": pc["xsT"], "deg_nm": pc["deg_nm"],
         "deg_row": pc["deg_row"],
         "wt": wt, "bias": bias, "kv": kv, "wg": wg, "ident": ident}
        for pc in per_core
    ]

    if os.environ.get("BASS_TRACE"):
        _install_ntff_hook()
    res = run_bass_kernel_spmd(nc, in_maps, core_ids=list(range(NC)))
    LAST_EXEC_TIME_NS = res.exec_time_ns

    out = np.empty((N, F), dtype=np.float32)
    for r in range(NC):
        lo = r * NPC
        hi = min(N, lo + NPC)
        if hi > lo:
            out[lo:hi] = res.results[r]["outT"].T[: hi - lo]
    return out


# revision 15
# speedup vs baseline: 1.4798x; 1.3626x over previous
"""Trainium2 Bass kernel for nn_PSN (gnn_message_passing), 8 NeuronCores.

Math (per reference):
    deg  = segment_sum(ones, col); deg[deg==0] = 1; dis = deg^-1/2
    repeat L times:  agg = scatter_add(col, dis[row]*dis[col]*cur[row]);
                     cur = cur - agg; update += tanh(k_i)*cur
    c = sigmoid(weighting); h = c*update + (1-c)*x; out = relu(h @ W.T + b)

Strategy (v4, quadrant-pipelined block-dense SpMM): target nodes are
sharded across 8 cores (1280/core); the adjacency is a dense grid of
128x128 count blocks in fp8 (exact small ints) streamed from HBM -- no
per-edge DMA.  Both the target dim and the source dim are split in half
and the layer loop is software-pipelined so each half-AllGather for the
next layer runs underneath the other half's matmuls:

  per layer:  MM(T0,S0) MM(T0,S1) | drain T0, y'_T0, AllGather_A(next)
              MM(T1,S0) MM(T1,S1) | drain T1, y'_T1, AllGather_B(next)

State is feature-major (curT [128 f, 1280 n]) so the final linear layer
needs no transposes; per-node scaling uses a pre-expanded disxT tensor.
"""
import sys
import types
import numpy as np
import ml_dtypes
from contextlib import ExitStack

import concourse.bass as bass
import concourse.tile as tile
from concourse import bacc, mybir
from concourse.bass_utils import run_bass_kernel_spmd

P = 128          # partitions / block size
NC = 8           # cores
F = 128          # feature dim (must equal P)
L = 8            # layers

FP32 = mybir.dt.float32
BF16 = mybir.dt.bfloat16
FP8 = mybir.dt.float8e4

LAST_EXEC_TIME_NS = None


def _install_ntff_hook():
    try:
        if "antenv.axon_hooks" in sys.modules:
            return
        import antenv
        from trn_agent_boot.trn_boot import _ntff_profile_via_ctypes

        m = types.ModuleType("antenv.axon_hooks")
        _state = {"hook": _ntff_profile_via_ctypes("/opt/axon/libaxon_pjrt.so")}
        m.set_axon_ntff_profile_hook = lambda h: _state.__setitem__("hook", h)
        m.get_axon_ntff_profile_hook = lambda: _state["hook"]
        sys.modules["antenv.axon_hooks"] = m
        antenv.axon_hooks = m
    except Exception:
        pass


def preprocess(x, edge_index):
    """Host-side index/layout prep: quadrant-ordered block-count adjacency
    (fp8), degree histogram, feature-major x slices."""
    N = x.shape[0]
    NB = -(-N // (NC * P))           # target blocks per core
    assert NB % 2 == 0
    NPC = NB * P
    NTOT = NC * NPC
    RB = NTOT // P
    HB = NB // 2
    RBH = RB // 2
    NPCh = NPC // 2

    row = np.asarray(edge_index[0], dtype=np.int64)
    col = np.asarray(edge_index[1], dtype=np.int64)

    A2 = np.zeros((NTOT, NTOT), dtype=np.int16)
    np.add.at(A2, (row, col), 1)

    deg = np.zeros(NTOT, dtype=np.float32)
    deg[:N] = np.bincount(col, minlength=N).astype(np.float32)

    x_pad = np.zeros((NTOT, F), dtype=np.float32)
    x_pad[:N] = np.asarray(x, dtype=np.float32)

    # source-block order: all ranks' first half-slices, then second halves
    rb_order = [r * NB + b for r in range(NC) for b in range(HB)] + \
               [r * NB + b for r in range(NC) for b in range(HB, NB)]

    per_core = []
    for r in range(NC):
        cs = slice(r * NPC, (r + 1) * NPC)
        a_blocks = A2[:, cs].reshape(RB, P, NPC)[rb_order]  # [rb, p, tgt]
        # [t-half, s-half, rb_local, p, tgt_local]
        a_q = np.ascontiguousarray(
            a_blocks.reshape(2, RBH, P, 2, NPCh).transpose(3, 0, 1, 2, 4))
        live = [[[rl for rl in range(RBH) if a_q[t, s, rl].any()]
                 for s in range(2)] for t in range(2)]
        a_r = a_q.astype(ml_dtypes.float8_e4m3)
        xsT = np.ascontiguousarray(x_pad[cs].T)            # [F, NPC]
        deg_nm = np.ascontiguousarray(deg[cs].reshape(NB, P).T)
        deg_row = np.ascontiguousarray(deg[cs].reshape(1, NPC))
        per_core.append({"a": a_r, "xsT": xsT, "deg_nm": deg_nm,
                         "deg_row": deg_row, "live": live})
    return per_core, NB, NPC, NTOT, N


def build_program(NB, live):
    """live[t][s] = list of non-zero source blocks (intersected over cores
    it must be a superset; we use the union so data is correct)."""
    NPC = NB * P
    NTOT = NC * NPC
    RB = NTOT // P
    HB = NB // 2
    RBH = RB // 2
    NPCh = NPC // 2
    HROWS = NC * HB * P
    RB_CHUNK = 8
    gsz = [512, NPCh - 512] if NPCh > 512 else [NPCh]
    ngr = len(gsz)

    nc = bacc.Bacc("TRN2", target_bir_lowering=False, debug=False,
                   enable_asserts=False, num_devices=NC)

    a_in = nc.dram_tensor("a", [2, 2, RBH, P, NPCh], FP8, kind="ExternalInput")
    x_in = nc.dram_tensor("xsT", [F, NPC], FP32, kind="ExternalInput")
    dnm_in = nc.dram_tensor("deg_nm", [P, NB], FP32, kind="ExternalInput")
    drow_in = nc.dram_tensor("deg_row", [1, NPC], FP32, kind="ExternalInput")
    wt_in = nc.dram_tensor("wt", [F, F], FP32, kind="ExternalInput")      # W.T
    bias_in = nc.dram_tensor("bias", [F, 1], FP32, kind="ExternalInput")
    kv_in = nc.dram_tensor("kv", [1, L], FP32, kind="ExternalInput")
    wg_in = nc.dram_tensor("wg", [1, 1], FP32, kind="ExternalInput")
    id_in = nc.dram_tensor("ident", [P, P], FP32, kind="ExternalInput")
    out_t = nc.dram_tensor("outT", [F, NPC], FP32, kind="ExternalOutput")

    with tile.TileContext(nc) as tc, ExitStack() as ctx:
        per = ctx.enter_context(tc.tile_pool(name="per", bufs=1))
        wk = ctx.enter_context(tc.tile_pool(name="wk", bufs=2))
        ps_agg = ctx.enter_context(tc.tile_pool(name="ps_agg", bufs=1, space="PSUM"))
        ps_m = ctx.enter_context(tc.tile_pool(name="ps_m", bufs=2, space="PSUM"))
        ps_f = ctx.enter_context(tc.tile_pool(name="ps_f", bufs=2, space="PSUM"))
        dram = ctx.enter_context(tc.tile_pool(name="dram", bufs=1, space="DRAM"))

        # persistent state (feature-major)
        xsT = per.tile([F, NPC], FP32)
        curT = per.tile([F, NPC], FP32)
        updT = per.tile([F, NPC], FP32)
        disxT = per.tile([F, NPC], FP32)
        dis_nm = per.tile([P, NB], FP32)
        y_sbA0 = per.tile([P, RBH * F], BF16)
        y_sbA1 = per.tile([P, RBH * F], BF16)
        y_sbB0 = per.tile([P, RBH * F], BF16)
        y_sbB1 = per.tile([P, RBH * F], BF16)
        y_sbAs = [y_sbA0, y_sbA1]
        y_sbBs = [y_sbB0, y_sbB1]
        yn = per.tile([P, NB * F], BF16)
        tanhk = per.tile([P, L], FP32)
        cbc = per.tile([P, 1], FP32)
        wt_sb = per.tile([F, F], FP32)
        id_sb = per.tile([P, P], FP32)
        bias_sb = per.tile([F, 1], FP32)
        ones1 = per.tile([1, P], FP32)
        a_sb = per.tile([P, 4 * RBH * NPCh], FP8)

        y_in = dram.tile([NPC, F], BF16)
        y_outA = nc.dram_tensor("y_outA_sh", [HROWS, F], BF16, addr_space="Shared").ap()
        y_outB = nc.dram_tensor("y_outB_sh", [HROWS, F], BF16, addr_space="Shared").ap()

        # ---- loads ----
        nc.sync.dma_start(
            a_sb[:].rearrange("p (q n) -> p q n", n=NPCh),
            a_in[:].rearrange("t s rl p n -> p (t s rl) n"))
        nc.sync.dma_start(xsT[:], x_in[:])
        nc.sync.dma_start(dis_nm[:], dnm_in[:])   # holds deg for now
        nc.sync.dma_start(wt_sb[:], wt_in[:])
        nc.sync.dma_start(id_sb[:], id_in[:])
        nc.sync.dma_start(bias_sb[:], bias_in[:])
        kv_sb = wk.tile([1, L], FP32)
        wg_sb = wk.tile([1, 1], FP32)
        drow = wk.tile([1, NPC], FP32)
        nc.sync.dma_start(kv_sb[:], kv_in[:])
        nc.sync.dma_start(wg_sb[:], wg_in[:])
        nc.sync.dma_start(drow[:], drow_in[:])

        nc.vector.memset(ones1[:], 1.0)
        nc.vector.memset(updT[:], 0.0)
        nc.vector.tensor_copy(curT[:], xsT[:])

        # ---- dis = (max(deg,1))^-1/2, both layouts ----
        nc.vector.tensor_scalar_max(dis_nm[:], dis_nm[:], 1.0)
        nc.vector.reciprocal(dis_nm[:], dis_nm[:])
        nc.scalar.activation(dis_nm[:], dis_nm[:], mybir.ActivationFunctionType.Sqrt)

        nc.vector.tensor_scalar_max(drow[:], drow[:], 1.0)
        nc.vector.reciprocal(drow[:], drow[:])
        nc.scalar.activation(drow[:], drow[:], mybir.ActivationFunctionType.Sqrt)
        for g in range(-(-NPC // 512)):
            w = min(512, NPC - g * 512)
            psd = ps_m.tile([P, 512], FP32, space="PSUM", tag="misc")
            nc.tensor.matmul(psd[:, :w], ones1[:],
                             drow[:, g * 512:g * 512 + w], start=True, stop=True)
            nc.vector.tensor_copy(disxT[:, g * 512:g * 512 + w], psd[:, :w])

        # ---- broadcast tanh(k) and sigmoid(weighting) ----
        psb = ps_m.tile([P, 512], FP32, space="PSUM", tag="misc")
        nc.tensor.matmul(psb[:, :L], ones1[:], kv_sb[:], start=True, stop=True)
        nc.scalar.activation(tanhk[:], psb[:, :L], mybir.ActivationFunctionType.Tanh)
        psb1 = ps_m.tile([P, 512], FP32, space="PSUM", tag="misc")
        nc.tensor.matmul(psb1[:, :1], ones1[:], wg_sb[:], start=True, stop=True)
        nc.scalar.activation(cbc[:], psb1[:, :1], mybir.ActivationFunctionType.Sigmoid)

        def yprod_and_ag(t, y_outH):
            """transpose+scale own target half t of curT into yn, DMA to
            y_in half, AllGather into y_outH."""
            for b in range(t * HB, (t + 1) * HB):
                pst = ps_f.tile([P, 512], FP32, space="PSUM", tag="fin")
                nc.tensor.transpose(pst[:, :P], curT[:, b * F:(b + 1) * F], id_sb[:])
                nc.vector.tensor_scalar_mul(
                    yn[:, b * F:(b + 1) * F], pst[:, :P], dis_nm[:, b:b + 1])
            hs = slice(t * HB * P, (t + 1) * HB * P)
            nc.sync.dma_start(
                y_in[hs, :].rearrange("(b p) f -> p b f", p=P),
                yn[:, t * HB * F:(t + 1) * HB * F].rearrange("p (b f) -> p b f", f=F))
            nc.gpsimd.collective_compute(
                "AllGather", mybir.AluOpType.bypass,
                replica_groups=[list(range(NC))],
                ins=[y_in[hs, :].opt()], outs=[y_outH[:].opt()])

        def load_ysb(y_sbH, y_outH):
            # chunked so matmuls can begin after the first chunk lands
            for c in range(RBH // RB_CHUNK):
                cw = RB_CHUNK * F
                nc.sync.dma_start(
                    y_sbH[:, c * cw:(c + 1) * cw].rearrange("p (rb f) -> p rb f", f=F),
                    y_outH[c * RB_CHUNK * P:(c + 1) * RB_CHUNK * P, :]
                    .rearrange("(rb p) f -> p rb f", p=P))

        # ---- initial y + AGs (parity 0) ----
        yprod_and_ag(0, y_outA)
        yprod_and_ag(1, y_outB)
        load_ysb(y_sbAs[0], y_outA)
        load_ysb(y_sbBs[0], y_outB)

        # ---- layers (quadrant pipeline, ping-pong half order) ----
        for i in range(L):
            t_first = i % 2
            s_first = 0 if i == 0 else (i - 1) % 2
            for t in (t_first, 1 - t_first):
                toff = t * NPCh
                psg = []
                for _g in range(ngr):
                    psgt = ps_agg.tile([P, 512], FP32, space="PSUM",
                                       tag=f"agg{t}{_g}")
                    psg.append(psgt)
                seq = [(s_first, rl) for rl in live[t][s_first]] + \
                      [(1 - s_first, rl) for rl in live[t][1 - s_first]]
                for qi, (s, rl) in enumerate(seq):
                    y_sbH = y_sbAs[i % 2] if s == 0 else y_sbBs[i % 2]
                    ao = ((t * 2 + s) * RBH + rl) * NPCh
                    for g in range(ngr):
                        go = sum(gsz[:g])
                        nc.tensor.matmul(
                            psg[g][:, :gsz[g]],
                            y_sbH[:, rl * F:(rl + 1) * F],
                            a_sb[:, ao + go: ao + go + gsz[g]],
                            start=(qi == 0), stop=(qi == len(seq) - 1))
                # drain target half t:  curT += psum * (-disxT)
                for g in range(ngr):
                    sl = slice(toff + sum(gsz[:g]), toff + sum(gsz[:g]) + gsz[g])
                    tmp = wk.tile([P, 512], FP32, tag="tmp")
                    nc.vector.tensor_tensor(tmp[:, :gsz[g]], psg[g][:, :gsz[g]],
                                            disxT[:, sl], mybir.AluOpType.mult)
                    nc.vector.tensor_tensor(curT[:, sl], curT[:, sl],
                                            tmp[:, :gsz[g]], mybir.AluOpType.subtract)
                ths = slice(toff, toff + NPCh)
                nc.vector.scalar_tensor_tensor(
                    updT[:, ths], curT[:, ths], tanhk[:, i:i + 1], updT[:, ths],
                    op0=mybir.AluOpType.mult, op1=mybir.AluOpType.add)
                if i < L - 1:
                    # next layer's half-AG + reload, hidden under other half's MMs
                    y_outH = y_outA if t == 0 else y_outB
                    y_sbH = (y_sbAs if t == 0 else y_sbBs)[(i + 1) % 2]
                    yprod_and_ag(t, y_outH)
                    load_ysb(y_sbH, y_outH)

        # ---- epilogue (in place: updT becomes h-temp, curT becomes h) ----
        nc.vector.tensor_tensor(updT[:], updT[:], xsT[:], mybir.AluOpType.subtract)
        nc.vector.scalar_tensor_tensor(
            curT[:], updT[:], cbc[:, 0:1], xsT[:],
            op0=mybir.AluOpType.mult, op1=mybir.AluOpType.add)
        for g in range(-(-NPC // 512)):
            w = min(512, NPC - g * 512)
            psf = ps_f.tile([P, 512], FP32, space="PSUM", tag="fin")
            nc.tensor.matmul(psf[:, :w], wt_sb[:],
                             curT[:, g * 512:g * 512 + w], start=True, stop=True)
            nc.scalar.activation(updT[:, g * 512:g * 512 + w], psf[:, :w],
                                 mybir.ActivationFunctionType.Relu,
                                 bias=bias_sb[:, 0:1])
        nc.sync.dma_start(out_t[:], updT[:])

    nc.compile()
    return nc


def kernel(x, edge_index, k_values, weighting, W, b):
    global LAST_EXEC_TIME_NS
    import os
    x = np.asarray(x)
    per_core, NB, NPC, NTOT, N = preprocess(x, edge_index)
    # a block may be skipped only if zero on every core
    live = [[sorted(set.union(*[set(pc["live"][t][s]) for pc in per_core]))
             for s in range(2)] for t in range(2)]
    nc = build_program(NB, live)

    wt = np.ascontiguousarray(np.asarray(W, dtype=np.float32).T)
    bias = np.asarray(b, dtype=np.float32).reshape(F, 1)
    kv = np.asarray(k_values, dtype=np.float32).reshape(1, L)
    wg = np.asarray(weighting, dtype=np.float32).reshape(1, 1)
    ident = np.eye(P, dtype=np.float32)

    in_maps = [
        {"a": pc["a"], "xsT": pc["xsT"], "deg_nm": pc["deg_nm"],
         "deg_row": pc["deg_row"],
         "wt": wt, "bias": bias, "kv": kv, "wg": wg, "ident": ident}
        for pc in per_core
    ]

    if os.environ.get("BASS_TRACE"):
        _install_ntff_hook()
    res = run_bass_kernel_spmd(nc, in_maps, core_ids=list(range(NC)))
    LAST_EXEC_TIME_NS = res.exec_time_ns

    out = np.empty((N, F), dtype=np.float32)
    for r in range(NC):
        lo = r * NPC
        hi = min(N, lo + NPC)
        if hi > lo:
            out[lo:hi] = res.results[r]["outT"].T[: hi - lo]
    return out


# revision 16
# speedup vs baseline: 1.4990x; 1.0129x over previous
"""Trainium2 Bass kernel for nn_PSN (gnn_message_passing), 8 NeuronCores.

Math (per reference):
    deg  = segment_sum(ones, col); deg[deg==0] = 1; dis = deg^-1/2
    repeat L times:  agg = scatter_add(col, dis[row]*dis[col]*cur[row]);
                     cur = cur - agg; update += tanh(k_i)*cur
    c = sigmoid(weighting); h = c*update + (1-c)*x; out = relu(h @ W.T + b)

Strategy (v4, quadrant-pipelined block-dense SpMM): target nodes are
sharded across 8 cores (1280/core); the adjacency is a dense grid of
128x128 count blocks in fp8 (exact small ints) streamed from HBM -- no
per-edge DMA.  Both the target dim and the source dim are split in half
and the layer loop is software-pipelined so each half-AllGather for the
next layer runs underneath the other half's matmuls:

  per layer:  MM(T0,S0) MM(T0,S1) | drain T0, y'_T0, AllGather_A(next)
              MM(T1,S0) MM(T1,S1) | drain T1, y'_T1, AllGather_B(next)

State is feature-major (curT [128 f, 1280 n]) so the final linear layer
needs no transposes; per-node scaling uses a pre-expanded disxT tensor.
"""
import sys
import types
import numpy as np
import ml_dtypes
from contextlib import ExitStack

import concourse.bass as bass
import concourse.tile as tile
from concourse import bacc, mybir
from concourse.bass_utils import run_bass_kernel_spmd

P = 128          # partitions / block size
NC = 8           # cores
F = 128          # feature dim (must equal P)
L = 8            # layers

FP32 = mybir.dt.float32
BF16 = mybir.dt.bfloat16
FP8 = mybir.dt.float8e4

LAST_EXEC_TIME_NS = None


def _install_ntff_hook():
    try:
        if "antenv.axon_hooks" in sys.modules:
            return
        import antenv
        from trn_agent_boot.trn_boot import _ntff_profile_via_ctypes

        m = types.ModuleType("antenv.axon_hooks")
        _state = {"hook": _ntff_profile_via_ctypes("/opt/axon/libaxon_pjrt.so")}
        m.set_axon_ntff_profile_hook = lambda h: _state.__setitem__("hook", h)
        m.get_axon_ntff_profile_hook = lambda: _state["hook"]
        sys.modules["antenv.axon_hooks"] = m
        antenv.axon_hooks = m
    except Exception:
        pass


def preprocess(x, edge_index):
    """Host-side index/layout prep: quadrant-ordered block-count adjacency
    (fp8), degree histogram, feature-major x slices."""
    N = x.shape[0]
    NB = -(-N // (NC * P))           # target blocks per core
    assert NB % 2 == 0
    NPC = NB * P
    NTOT = NC * NPC
    RB = NTOT // P
    HB = NB // 2
    RBH = RB // 2
    NPCh = NPC // 2

    row = np.asarray(edge_index[0], dtype=np.int64)
    col = np.asarray(edge_index[1], dtype=np.int64)

    A2 = np.zeros((NTOT, NTOT), dtype=np.int16)
    np.add.at(A2, (row, col), 1)

    deg = np.zeros(NTOT, dtype=np.float32)
    deg[:N] = np.bincount(col, minlength=N).astype(np.float32)

    x_pad = np.zeros((NTOT, F), dtype=np.float32)
    x_pad[:N] = np.asarray(x, dtype=np.float32)

    # source-block order: all ranks' first half-slices, then second halves
    rb_order = [r * NB + b for r in range(NC) for b in range(HB)] + \
               [r * NB + b for r in range(NC) for b in range(HB, NB)]

    per_core = []
    for r in range(NC):
        cs = slice(r * NPC, (r + 1) * NPC)
        a_blocks = A2[:, cs].reshape(RB, P, NPC)[rb_order]  # [rb, p, tgt]
        # [t-half, s-half, rb_local, p, tgt_local]
        a_q = np.ascontiguousarray(
            a_blocks.reshape(2, RBH, P, 2, NPCh).transpose(3, 0, 1, 2, 4))
        live = [[[rl for rl in range(RBH) if a_q[t, s, rl].any()]
                 for s in range(2)] for t in range(2)]
        a_r = a_q.astype(ml_dtypes.float8_e4m3)
        xsT = np.ascontiguousarray(x_pad[cs].T)            # [F, NPC]
        deg_nm = np.ascontiguousarray(deg[cs].reshape(NB, P).T)
        deg_row = np.ascontiguousarray(deg[cs].reshape(1, NPC))
        per_core.append({"a": a_r, "xsT": xsT, "deg_nm": deg_nm,
                         "deg_row": deg_row, "live": live})
    return per_core, NB, NPC, NTOT, N


def build_program(NB, live):
    """live[t][s] = list of non-zero source blocks (intersected over cores
    it must be a superset; we use the union so data is correct)."""
    NPC = NB * P
    NTOT = NC * NPC
    RB = NTOT // P
    HB = NB // 2
    RBH = RB // 2
    NPCh = NPC // 2
    HROWS = NC * HB * P
    RB_CHUNK = 8
    gsz = [512, NPCh - 512] if NPCh > 512 else [NPCh]
    ngr = len(gsz)

    nc = bacc.Bacc("TRN2", target_bir_lowering=False, debug=False,
                   enable_asserts=False, num_devices=NC)

    a_in = nc.dram_tensor("a", [2, 2, RBH, P, NPCh], FP8, kind="ExternalInput")
    x_in = nc.dram_tensor("xsT", [F, NPC], FP32, kind="ExternalInput")
    dnm_in = nc.dram_tensor("deg_nm", [P, NB], FP32, kind="ExternalInput")
    drow_in = nc.dram_tensor("deg_row", [1, NPC], FP32, kind="ExternalInput")
    wt_in = nc.dram_tensor("wt", [F, F], FP32, kind="ExternalInput")      # W.T
    bias_in = nc.dram_tensor("bias", [F, 1], FP32, kind="ExternalInput")
    kv_in = nc.dram_tensor("kv", [1, L], FP32, kind="ExternalInput")
    wg_in = nc.dram_tensor("wg", [1, 1], FP32, kind="ExternalInput")
    id_in = nc.dram_tensor("ident", [P, P], FP32, kind="ExternalInput")
    out_t = nc.dram_tensor("outT", [F, NPC], FP32, kind="ExternalOutput")

    with tile.TileContext(nc) as tc, ExitStack() as ctx:
        per = ctx.enter_context(tc.tile_pool(name="per", bufs=1))
        wk = ctx.enter_context(tc.tile_pool(name="wk", bufs=2))
        ps_agg = ctx.enter_context(tc.tile_pool(name="ps_agg", bufs=1, space="PSUM"))
        ps_m = ctx.enter_context(tc.tile_pool(name="ps_m", bufs=2, space="PSUM"))
        ps_f = ctx.enter_context(tc.tile_pool(name="ps_f", bufs=2, space="PSUM"))
        dram = ctx.enter_context(tc.tile_pool(name="dram", bufs=1, space="DRAM"))

        # persistent state (feature-major)
        xsT = per.tile([F, NPC], FP32)
        curT = per.tile([F, NPC], FP32)
        updT = per.tile([F, NPC], FP32)
        disxT = per.tile([F, NPC], FP32)
        dis_nm = per.tile([P, NB], FP32)
        y_sbA0 = per.tile([P, RBH * F], BF16)
        y_sbA1 = per.tile([P, RBH * F], BF16)
        y_sbB0 = per.tile([P, RBH * F], BF16)
        y_sbB1 = per.tile([P, RBH * F], BF16)
        y_sbAs = [y_sbA0, y_sbA1]
        y_sbBs = [y_sbB0, y_sbB1]
        yn = per.tile([P, NB * F], BF16)
        tanhk = per.tile([P, L], FP32)
        cbc = per.tile([P, 1], FP32)
        wt_sb = per.tile([F, F], FP32)
        id_sb = per.tile([P, P], FP32)
        bias_sb = per.tile([F, 1], FP32)
        ones1 = per.tile([1, P], FP32)
        a_sb = per.tile([P, 4 * RBH * NPCh], FP8)

        y_in = dram.tile([NPC, F], BF16)
        y_outA = nc.dram_tensor("y_outA_sh", [HROWS, F], BF16, addr_space="Shared").ap()
        y_outB = nc.dram_tensor("y_outB_sh", [HROWS, F], BF16, addr_space="Shared").ap()

        # ---- loads ----
        nc.sync.dma_start(xsT[:], x_in[:])
        nc.sync.dma_start(dis_nm[:], dnm_in[:])   # holds deg for now
        nc.sync.dma_start(wt_sb[:], wt_in[:])
        nc.sync.dma_start(id_sb[:], id_in[:])
        nc.sync.dma_start(bias_sb[:], bias_in[:])
        kv_sb = wk.tile([1, L], FP32)
        wg_sb = wk.tile([1, 1], FP32)
        drow = wk.tile([1, NPC], FP32)
        nc.sync.dma_start(kv_sb[:], kv_in[:])
        nc.sync.dma_start(wg_sb[:], wg_in[:])
        nc.sync.dma_start(drow[:], drow_in[:])

        nc.vector.memset(ones1[:], 1.0)
        nc.vector.memset(updT[:], 0.0)
        nc.vector.tensor_copy(curT[:], xsT[:])

        # ---- dis = (max(deg,1))^-1/2, both layouts ----
        nc.vector.tensor_scalar_max(dis_nm[:], dis_nm[:], 1.0)
        nc.vector.reciprocal(dis_nm[:], dis_nm[:])
        nc.scalar.activation(dis_nm[:], dis_nm[:], mybir.ActivationFunctionType.Sqrt)

        nc.vector.tensor_scalar_max(drow[:], drow[:], 1.0)
        nc.vector.reciprocal(drow[:], drow[:])
        nc.scalar.activation(drow[:], drow[:], mybir.ActivationFunctionType.Sqrt)
        for g in range(-(-NPC // 512)):
            w = min(512, NPC - g * 512)
            psd = ps_m.tile([P, 512], FP32, space="PSUM", tag="misc")
            nc.tensor.matmul(psd[:, :w], ones1[:],
                             drow[:, g * 512:g * 512 + w], start=True, stop=True)
            nc.vector.tensor_copy(disxT[:, g * 512:g * 512 + w], psd[:, :w])

        # ---- broadcast tanh(k) and sigmoid(weighting) ----
        psb = ps_m.tile([P, 512], FP32, space="PSUM", tag="misc")
        nc.tensor.matmul(psb[:, :L], ones1[:], kv_sb[:], start=True, stop=True)
        nc.scalar.activation(tanhk[:], psb[:, :L], mybir.ActivationFunctionType.Tanh)
        psb1 = ps_m.tile([P, 512], FP32, space="PSUM", tag="misc")
        nc.tensor.matmul(psb1[:, :1], ones1[:], wg_sb[:], start=True, stop=True)
        nc.scalar.activation(cbc[:], psb1[:, :1], mybir.ActivationFunctionType.Sigmoid)

        def yprod_and_ag(t, y_outH):
            """transpose+scale own target half t of curT into yn, DMA to
            y_in half, AllGather into y_outH."""
            for b in range(t * HB, (t + 1) * HB):
                pst = ps_f.tile([P, 512], FP32, space="PSUM", tag="fin")
                nc.tensor.transpose(pst[:, :P], curT[:, b * F:(b + 1) * F], id_sb[:])
                nc.vector.tensor_scalar_mul(
                    yn[:, b * F:(b + 1) * F], pst[:, :P], dis_nm[:, b:b + 1])
            hs = slice(t * HB * P, (t + 1) * HB * P)
            nc.sync.dma_start(
                y_in[hs, :].rearrange("(b p) f -> p b f", p=P),
                yn[:, t * HB * F:(t + 1) * HB * F].rearrange("p (b f) -> p b f", f=F))
            nc.gpsimd.collective_compute(
                "AllGather", mybir.AluOpType.bypass,
                replica_groups=[list(range(NC))],
                ins=[y_in[hs, :].opt()], outs=[y_outH[:].opt()])

        def load_ysb(y_sbH, y_outH):
            # chunked so matmuls can begin after the first chunk lands
            for c in range(RBH // RB_CHUNK):
                cw = RB_CHUNK * F
                nc.sync.dma_start(
                    y_sbH[:, c * cw:(c + 1) * cw].rearrange("p (rb f) -> p rb f", f=F),
                    y_outH[c * RB_CHUNK * P:(c + 1) * RB_CHUNK * P, :]
                    .rearrange("(rb p) f -> p rb f", p=P))

        # ---- initial y + AGs (parity 0) ----
        yprod_and_ag(0, y_outA)
        yprod_and_ag(1, y_outB)
        load_ysb(y_sbAs[0], y_outA)
        load_ysb(y_sbBs[0], y_outB)
        # A loads, in first-use order (t0s0, t0s1, t1s0, t1s1), issued after
        # the collectives so small prologue DMAs aren't stuck behind 13MB
        for (t_, s_) in ((0, 0), (0, 1), (1, 0), (1, 1)):
            q = t_ * 2 + s_
            nc.sync.dma_start(
                a_sb[:, q * RBH * NPCh:(q + 1) * RBH * NPCh]
                .rearrange("p (rl n) -> p rl n", n=NPCh),
                a_in[t_, s_, :, :, :].rearrange("rl p n -> p rl n"))

        # ---- layers (quadrant pipeline, ping-pong half order) ----
        for i in range(L):
            t_first = i % 2
            s_first = 0 if i == 0 else (i - 1) % 2
            for t in (t_first, 1 - t_first):
                toff = t * NPCh
                psg = []
                for _g in range(ngr):
                    psgt = ps_agg.tile([P, 512], FP32, space="PSUM",
                                       tag=f"agg{t}{_g}")
                    psg.append(psgt)
                seq = [(s_first, rl) for rl in live[t][s_first]] + \
                      [(1 - s_first, rl) for rl in live[t][1 - s_first]]
                for qi, (s, rl) in enumerate(seq):
                    y_sbH = y_sbAs[i % 2] if s == 0 else y_sbBs[i % 2]
                    ao = ((t * 2 + s) * RBH + rl) * NPCh
                    for g in range(ngr):
                        go = sum(gsz[:g])
                        nc.tensor.matmul(
                            psg[g][:, :gsz[g]],
                            y_sbH[:, rl * F:(rl + 1) * F],
                            a_sb[:, ao + go: ao + go + gsz[g]],
                            start=(qi == 0), stop=(qi == len(seq) - 1))
                # drain target half t:  curT += psum * (-disxT)
                for g in range(ngr):
                    sl = slice(toff + sum(gsz[:g]), toff + sum(gsz[:g]) + gsz[g])
                    tmp = wk.tile([P, 512], FP32, tag="tmp")
                    nc.vector.tensor_tensor(tmp[:, :gsz[g]], psg[g][:, :gsz[g]],
                                            disxT[:, sl], mybir.AluOpType.mult)
                    nc.vector.tensor_tensor(curT[:, sl], curT[:, sl],
                                            tmp[:, :gsz[g]], mybir.AluOpType.subtract)
                ths = slice(toff, toff + NPCh)
                nc.vector.scalar_tensor_tensor(
                    updT[:, ths], curT[:, ths], tanhk[:, i:i + 1], updT[:, ths],
                    op0=mybir.AluOpType.mult, op1=mybir.AluOpType.add)
                if i < L - 1:
                    # next layer's half-AG + reload, hidden under other half's MMs
                    y_outH = y_outA if t == 0 else y_outB
                    y_sbH = (y_sbAs if t == 0 else y_sbBs)[(i + 1) % 2]
                    yprod_and_ag(t, y_outH)
                    load_ysb(y_sbH, y_outH)

        # ---- epilogue (in place: updT becomes h-temp, curT becomes h) ----
        nc.vector.tensor_tensor(updT[:], updT[:], xsT[:], mybir.AluOpType.subtract)
        nc.vector.scalar_tensor_tensor(
            curT[:], updT[:], cbc[:, 0:1], xsT[:],
            op0=mybir.AluOpType.mult, op1=mybir.AluOpType.add)
        for g in range(-(-NPC // 512)):
            w = min(512, NPC - g * 512)
            psf = ps_f.tile([P, 512], FP32, space="PSUM", tag="fin")
            nc.tensor.matmul(psf[:, :w], wt_sb[:],
                             curT[:, g * 512:g * 512 + w], start=True, stop=True)
            nc.scalar.activation(updT[:, g * 512:g * 512 + w], psf[:, :w],
                                 mybir.ActivationFunctionType.Relu,
                                 bias=bias_sb[:, 0:1])
        nc.sync.dma_start(out_t[:], updT[:])

    nc.compile()
    return nc


def kernel(x, edge_index, k_values, weighting, W, b):
    global LAST_EXEC_TIME_NS
    import os
    x = np.asarray(x)
    per_core, NB, NPC, NTOT, N = preprocess(x, edge_index)
    # a block may be skipped only if zero on every core
    live = [[sorted(set.union(*[set(pc["live"][t][s]) for pc in per_core]))
             for s in range(2)] for t in range(2)]
    nc = build_program(NB, live)

    wt = np.ascontiguousarray(np.asarray(W, dtype=np.float32).T)
    bias = np.asarray(b, dtype=np.float32).reshape(F, 1)
    kv = np.asarray(k_values, dtype=np.float32).reshape(1, L)
    wg = np.asarray(weighting, dtype=np.float32).reshape(1, 1)
    ident = np.eye(P, dtype=np.float32)

    in_maps = [
        {"a": pc["a"], "xsT": pc["xsT"], "deg_nm": pc["deg_nm"],
         "deg_row": pc["deg_row"],
         "wt": wt, "bias": bias, "kv": kv, "wg": wg, "ident": ident}
        for pc in per_core
    ]

    if os.environ.get("BASS_TRACE"):
        _install_ntff_hook()
    res = run_bass_kernel_spmd(nc, in_maps, core_ids=list(range(NC)))
    LAST_EXEC_TIME_NS = res.exec_time_ns

    out = np.empty((N, F), dtype=np.float32)
    for r in range(NC):
        lo = r * NPC
        hi = min(N, lo + NPC)
        if hi > lo:
            out[lo:hi] = res.results[r]["outT"].T[: hi - lo]
    return out
